# revision 20
# baseline (speedup 1.0000x reference)
"""Trainium2 Bass kernel for a cached-attention block (LN + RoPE + MHA).

Sharding over 8 cores: data-parallel over batch (4) x tensor-parallel over
heads (16 -> 2 groups of 8). Core c handles batch c//2, head-group c%2.
Each core computes a partial output projection (its 8 heads); the two
partials per batch are summed on the host (the all-reduce of the hint).

Per-core dataflow (S=2048, D=1024, 8 local heads, head dim 64):
  Phase 1 (one merged software-pipelined loop over 16 s-chunks):
    LN stats via bn_stats (DVE) + sqrt (ACT) + reciprocal (DVE);
    normalize on ACT (Identity, scale=rstd, bias=-mu*rstd from Pool);
    RoPE on q/k split between DVE (first half) and Pool (second half);
    PE transpose to [d, s]; projections on PE -> qhT/khT [hk, s] fp16 and
    vh [sk, hv] fp16 with a leading all-ones column per head.
  Phase 2 (attention, per q-half then head-pair):
    scores^T [sk, q] fp16 PSUM = khT.T @ qhT (K=64, two heads row-packed
    via tile_position auto-derive); exp on ACT (the phase bottleneck);
    ctx^T [65, q] fp32 accumulated over sk chunks, row 0 = prob sums;
    reciprocal of sums (fast DVE op, directly from PSUM), DMA broadcast,
    normalize during evacuation (DVE).
    After each q-half's 4 head-pairs: output projection for those 8
    s-chunks on PE, DMA straight from PSUM -> overlaps the other
    q-half's ACT-bound attention.
"""

import numpy as np

S = 2048
D = 1024
H_LOC = 8  # heads per core
HK = H_LOC * 64  # 512
N_CORES = 8
EPS = 1e-6
ROPE_BASE = 10000.0

_cached = {}


def _build_program(dbg=False, loop_k=None, ph=(1, 1, 1, 1)):
    import contextlib

    import concourse.tile as tile
    from concourse import bacc, mybir

    f32 = mybir.dt.float32
    f16 = mybir.dt.float16
    AF = mybir.ActivationFunctionType
    OP = mybir.AluOpType

    nc = bacc.Bacc("TRN2", target_bir_lowering=False, debug=False,
                   num_devices=N_CORES)

    xq_d = nc.dram_tensor("xq", [S, D], f32, kind="ExternalInput").ap()
    xk_d = nc.dram_tensor("xk", [S, D], f32, kind="ExternalInput").ap()
    xv_d = nc.dram_tensor("xv", [S, D], f32, kind="ExternalInput").ap()
    wq_d = nc.dram_tensor("wq", [128, 8, HK], f16, kind="ExternalInput").ap()
    wk_d = nc.dram_tensor("wk", [128, 8, HK], f16, kind="ExternalInput").ap()
    wv_d = nc.dram_tensor("wv", [128, 8, HK], f16, kind="ExternalInput").ap()
    wo_d = nc.dram_tensor("wo", [128, 4, D], f16, kind="ExternalInput").ap()
    cos_d = nc.dram_tensor("cost", [S, D // 2], f16, kind="ExternalInput").ap()
    sin_d = nc.dram_tensor("sint", [S, D // 2], f16, kind="ExternalInput").ap()
    id_d = nc.dram_tensor("ident", [128, 128], f16, kind="ExternalInput").ap()
    out_d = nc.dram_tensor("out", [S, D], f32, kind="ExternalOutput").ap()
    if dbg:
        dbg_qhT = nc.dram_tensor("dbg_qhT", [128, 4, S], f16,
                                 kind="ExternalOutput").ap()
        dbg_khT = nc.dram_tensor("dbg_khT", [128, 4, S], f16,
                                 kind="ExternalOutput").ap()
        dbg_vh = nc.dram_tensor("dbg_vh", [128, 16, H_LOC * 65], f16,
                                kind="ExternalOutput").ap()
        dbg_ctxT = nc.dram_tensor("dbg_ctxT", [128, 4, S], f16,
                                  kind="ExternalOutput").ap()
        dbg_pr = nc.dram_tensor("dbg_pr", [128, 16, 1024], f16,
                                kind="ExternalOutput").ap()
        dbg_qT = nc.dram_tensor("dbg_qT", [128, 8, 512], f16,
                                kind="ExternalOutput").ap()
        dbg_ctxu = nc.dram_tensor("dbg_ctxu", [65, 1024], f32,
                                  kind="ExternalOutput").ap()
        dbg_rs = nc.dram_tensor("dbg_rs", [1, 1024], f32,
                                kind="ExternalOutput").ap()

    with tile.TileContext(nc) as tc:
        with tc.tile_pool(name="persist", bufs=1) as P:
            # --- persistent SBUF ---
            wq_sb = P.tile([128, 8, HK], f16, tag="wq")
            wk_sb = P.tile([128, 8, HK], f16, tag="wk")
            wv_sb = P.tile([128, 8, HK], f16, tag="wv")
            wo_sb = P.tile([128, 4, D], f16, tag="wo")
            nc.sync.dma_start(wq_sb[:], wq_d[:])
            nc.sync.dma_start(wk_sb[:], wk_d[:])
            nc.sync.dma_start(wv_sb[:], wv_d[:])
            nc.sync.dma_start(wo_sb[:], wo_d[:])

            qhT = P.tile([128, 4, S], f16, tag="qhT")   # [hk, s] head h: part 64*(h%2), chunk h//2
            khT = P.tile([128, 4, S], f16, tag="khT")
            vh = P.tile([128, 16, H_LOC * 65], f16, tag="vh")  # [sk, skc, h*65+v]; col h*65+0 = 1
            ctxT = P.tile([128, 4, S], f16, tag="ctxT")
            eps_sb = P.tile([128, 1], f32, tag="eps")
            nc.vector.memset(eps_sb[:], EPS)
            id_sb = P.tile([128, 128], f16, tag="ident")
            nc.sync.dma_start(id_sb[:], id_d[:])
            vh3 = vh[:].rearrange("p c (h u) -> p c h u", u=65)
            for h in range(H_LOC):
                nc.vector.memset(vh3[:, :, h, 64:65], 1.0)

            if not ph[0]:
                # phase-isolated benches: init tensors phase 1 would write
                nc.vector.memset(qhT[:], 0.01)
                nc.vector.memset(khT[:], 0.01)
                nc.vector.memset(ctxT[:], 0.01)

            loop_cm = (tc.For_i(0, loop_k, 1) if loop_k is not None
                       else contextlib.nullcontext())

            with loop_cm, tc.tile_pool(name="slab", bufs=2) as SL:
                slabs = {}
                if ph[0]:
                    # ---------------- Phase 1: merged q/k/v pipeline --------
                    with tc.tile_pool(name="ph1x", bufs=2) as X, \
                         tc.tile_pool(name="ph1", bufs=3) as Q, \
                         tc.tile_pool(name="ps1x", bufs=2, space="PSUM") as PSX, \
                         tc.tile_pool(name="ps1p", bufs=2, space="PSUM") as PSP:

                        def ln_stats(x_t, tag):
                            stats = Q.tile([128, 2, 6], f32, tag=f"st_{tag}",
                                           name="st")
                            mv = Q.tile([128, 2], f32, tag=f"mv_{tag}",
                                        name="mv")
                            sq = Q.tile([128, 1], f32, tag=f"sq_{tag}",
                                        name="sq")
                            rstd = Q.tile([128, 1], f32, tag=f"rs_{tag}",
                                          name="rstd")
                            nmr = Q.tile([128, 1], f32, tag=f"nm_{tag}",
                                         name="nmr")
                            nc.vector.bn_stats(stats[:, 0, :], x_t[:, 0:512])
                            nc.vector.bn_stats(stats[:, 1, :], x_t[:, 512:1024])
                            nc.vector.bn_aggr(mv[:], stats[:])
                            nc.scalar.activation(sq[:], mv[:, 1:2], AF.Sqrt,
                                                 bias=eps_sb[:])
                            nc.vector.reciprocal(rstd[:], sq[:])
                            nc.gpsimd.tensor_scalar(
                                nmr[:], mv[:, 0:1], rstd[:], -1.0,
                                OP.mult, OP.mult)
                            return rstd, nmr

                        def normalize(x_t, out_t, rstd, nmr):
                            nc.scalar.activation(out_t[:], x_t[:], AF.Identity,
                                                 bias=nmr[:], scale=rstd[:])

                        def project(sg):
                            qTg, kTg = slabs.pop(sg)
                            if dbg and sg == 0:
                                nc.sync.dma_start(dbg_qT[:], qTg[:])
                            for hkc in range(4):
                                for w_sb, src, dst in ((wq_sb, qTg, qhT),
                                                       (wk_sb, kTg, khT)):
                                    ps = PSP.tile([128, 512], f32, tag="pj",
                                                  name="pj")
                                    for dc in range(8):
                                        nc.tensor.matmul(
                                            ps[:],
                                            lhsT=w_sb[:, dc, hkc * 128:(hkc + 1) * 128],
                                            rhs=src[:, dc, :],
                                            start=(dc == 0), stop=(dc == 7))
                                    nc.scalar.copy(
                                        dst[:, hkc, sg * 512:(sg + 1) * 512],
                                        ps[:])

                        prev = None
                        for i in range(17):
                            cur = None
                            if i < 16:
                                if i % 4 == 0:
                                    slabs[i // 4] = (
                                        SL.tile([128, 8, 512], f16, tag="qTg",
                                                name="qTg"),
                                        SL.tile([128, 8, 512], f16, tag="kTg",
                                                name="kTg"))
                                cos_t = Q.tile([128, 512], f16, tag="cos")
                                sin_t = Q.tile([128, 512], f16, tag="sin")
                                nc.sync.dma_start(cos_t[:], cos_d[i * 128:(i + 1) * 128, :])
                                nc.sync.dma_start(sin_t[:], sin_d[i * 128:(i + 1) * 128, :])
                                cur = {"cos": cos_t, "sin": sin_t, "i": i}
                                for nm, x_d in (("q", xq_d), ("k", xk_d),
                                                ("v", xv_d)):
                                    x_t = X.tile([128, D], f32, tag=f"x{nm}",
                                                 name="x_t")
                                    nc.sync.dma_start(x_t[:], x_d[i * 128:(i + 1) * 128, :])
                                    rstd, nmr = ln_stats(x_t, nm)
                                    cur[nm] = (x_t, rstd, nmr)
                            if prev is not None:
                                pi = prev["i"]
                                psg, pst = pi // 4, pi % 4
                                # ---- v path ----
                                x_t, rstd, nmr = prev["v"]
                                vn_t = X.tile([128, D], f16, tag="nv",
                                              name="vn_t")
                                normalize(x_t, vn_t, rstd, nmr)
                                xp_ps = PSX.tile([128, 8, 128], f16, tag="xpv",
                                                 name="xp_v")
                                for dc in range(8):
                                    nc.tensor.transpose(
                                        xp_ps[:, dc, :],
                                        vn_t[:, dc * 128:(dc + 1) * 128],
                                        id_sb[:])
                                vT_t = X.tile([128, 8, 128], f16, tag="vT",
                                              name="vT_t")
                                nc.vector.tensor_copy(vT_t[:], xp_ps[:])
                                vp = PSP.tile([128, HK], f32, tag="pj",
                                              name="vp")
                                for dc in range(8):
                                    nc.tensor.matmul(vp[:], lhsT=vT_t[:, dc, :],
                                                     rhs=wv_sb[:, dc, :],
                                                     start=(dc == 0),
                                                     stop=(dc == 7))
                                nc.scalar.copy(
                                    vh3[:, pi, :, 0:64],
                                    vp[:].rearrange("p (h v) -> p h v", v=64))
                                # ---- q/k path ----
                                xps = {}
                                for nm in ("q", "k"):
                                    x_t, rstd, nmr = prev[nm]
                                    n_t = Q.tile([128, D], f16, tag=f"n{nm}",
                                                 name="n_t")
                                    normalize(x_t, n_t, rstd, nmr)
                                    r_t = Q.tile([128, D], f16, tag=f"r{nm}",
                                                 name="r_t")
                                    b = Q.tile([128, 512], f16, tag=f"b{nm}",
                                               name="b")
                                    d = Q.tile([128, 512], f16, tag=f"d{nm}",
                                               name="d")
                                    # first rope half on DVE (in-place sub)
                                    nc.vector.tensor_mul(
                                        r_t[:, 0:512], n_t[:, 0:512],
                                        prev["cos"][:])
                                    nc.vector.tensor_mul(
                                        b[:], n_t[:, 512:1024], prev["sin"][:])
                                    nc.vector.tensor_sub(
                                        r_t[:, 0:512], r_t[:, 0:512], b[:])
                                    # second rope half: muls on Pool,
                                    # final add on DVE
                                    nc.gpsimd.tensor_mul(
                                        r_t[:, 512:1024], n_t[:, 512:1024],
                                        prev["cos"][:])
                                    nc.gpsimd.tensor_mul(
                                        d[:], n_t[:, 0:512], prev["sin"][:])
                                    nc.vector.tensor_add(
                                        r_t[:, 512:1024], r_t[:, 512:1024],
                                        d[:])
                                    xq_ps = PSX.tile([128, 8, 128], f16,
                                                     tag=f"xp{nm}", name="xq_ps")
                                    for dc in range(8):
                                        nc.tensor.transpose(
                                            xq_ps[:, dc, :],
                                            r_t[:, dc * 128:(dc + 1) * 128],
                                            id_sb[:])
                                    xps[nm] = xq_ps
                                for nm, dst_i in (("q", 0), ("k", 1)):
                                    nc.scalar.copy(
                                        slabs[psg][dst_i][:, :, pst * 128:(pst + 1) * 128],
                                        xps[nm][:])
                                if pst == 3 and psg < 3:
                                    # sg3's projections are deferred into
                                    # phase 2 (hidden under ACT-bound passes)
                                    project(psg)
                            prev = cur

                if ph[2]:
                    # ---------------- Phase 2: attention + out-proj ---------
                    import concourse.bass as bass
                    with tc.tile_pool(name="ph2", bufs=8) as PR, \
                         tc.tile_pool(name="ph2b", bufs=2) as RS, \
                         tc.tile_pool(name="ps2", bufs=1, space="PSUM") as PS2:
                        for qc in range(2):
                            q0 = qc * 1024
                            for hp in range(4):
                                h0, h1 = 2 * hp, 2 * hp + 1
                                if qc == 0 and ph[0]:
                                    # deferred sg3 projection for this
                                    # head-pair (hkc == hp), scratch in the
                                    # scores tag, evac on DVE
                                    qTg3, kTg3 = slabs[3]
                                    for w_sb, src3, dst in ((wq_sb, qTg3, qhT),
                                                            (wk_sb, kTg3, khT)):
                                        pps = PS2.tile([128, 1024], f32,
                                                       tag="sc0", name="pj3")
                                        for dc in range(8):
                                            nc.tensor.matmul(
                                                pps[:, 0:512],
                                                lhsT=w_sb[:, dc, hp * 128:(hp + 1) * 128],
                                                rhs=src3[:, dc, :],
                                                start=(dc == 0), stop=(dc == 7))
                                        nc.vector.tensor_copy(
                                            dst[:, hp, 3 * 512:4 * 512],
                                            pps[:, 0:512])
                                ctx_ps = [PS2.tile([65, 1024], f32, tag=f"ctx{e}",
                                                    name=f"ctx{e}") for e in range(2)]
                                # software pipeline: ctx matmuls for chunk
                                # skc-2 are issued after the scores/exp of
                                # chunk skc, so the PE never sits directly
                                # behind the exp it is waiting for.
                                hist = {}
                                for skc in range(18):
                                    cur = [None, None]
                                    if skc < 16:
                                        st_ps = [PS2.tile([128, 1024], f32, tag=f"sc{e}",
                                                           name=f"sc{e}") for e in range(2)]
                                        for j in range(2):
                                            for e, h in ((0, h0), (1, h1)):
                                                pb = slice(64 * (h % 2), 64 * (h % 2) + 64)
                                                nc.tensor.matmul(
                                                    st_ps[e][:, j * 512:(j + 1) * 512],
                                                    lhsT=khT[pb, hp, skc * 128:(skc + 1) * 128],
                                                    rhs=qhT[pb, hp, q0 + j * 512:q0 + (j + 1) * 512])
                                        for e, h in ((0, h0), (1, h1)):
                                            pr = PR.tile([128, 1024], f16, tag=f"pr{e}")
                                            nc.scalar.activation(pr[:], st_ps[e][:], AF.Exp,
                                                                 scale=0.125)
                                            if dbg and hp == 0 and qc == 0 and e == 0:
                                                nc.sync.dma_start(dbg_pr[:, skc, :], pr[:])
                                            cur[e] = pr
                                        hist[skc] = cur
                                    pskc = skc - 2
                                    if pskc >= 0:
                                        pcur = hist.pop(pskc)
                                        for e, h in ((0, h0), (1, h1)):
                                            lw = vh3[:, pskc, h, 0:65]
                                            for j in range(2):
                                                nc.tensor.matmul(
                                                    ctx_ps[e][:, j * 512:(j + 1) * 512],
                                                    lhsT=lw,
                                                    rhs=pcur[e][:, j * 512:(j + 1) * 512],
                                                    start=(pskc == 0), stop=(pskc == 15))
                                if dbg and hp == 0 and qc == 0:
                                    ctxu_sb = RS.tile([65, 1024], f32, tag="dbgu")
                                    nc.vector.tensor_copy(ctxu_sb[:], ctx_ps[0][:])
                                    nc.sync.dma_start(dbg_ctxu[:], ctxu_sb[:])
                                for e, h in ((0, h0), (1, h1)):
                                    # sums row (partition 64) staged to
                                    # partition 0 for the custom DVE recip;
                                    # the normalize mul reads ctx PSUM
                                    # directly during evacuation.
                                    s_sb = RS.tile([1, 1024], f32, tag="ssum")
                                    nc.vector.tensor_copy(s_sb[:], ctx_ps[e][64:65, :])
                                    cu = RS.tile([64, 1024], f16, tag="cu")
                                    nc.vector.tensor_copy(cu[:], ctx_ps[e][0:64, :])
                                    rs = RS.tile([1, 1024], f32, tag="rs")
                                    nc.vector.reciprocal_approx_fast(
                                        out=rs[:], in_=s_sb[:])
                                    rs16 = RS.tile([1, 1024], f16, tag="rs16")
                                    nc.vector.tensor_copy(rs16[:], rs[:])
                                    rsb = RS.tile([64, 1024], f16, tag="rsb")
                                    nc.gpsimd.partition_broadcast(
                                        rsb[:], rs16[:], channels=64)
                                    if dbg and hp == 0 and qc == 0 and e == 0:
                                        nc.sync.dma_start(dbg_rs[:], rs[:])
                                    nc.vector.tensor_mul(
                                        ctxT[64 * (h % 2):64 * (h % 2) + 64, h // 2,
                                             q0:q0 + 1024],
                                        cu[:], rsb[:])
                    # ---------------- Phase 3: output projection ----------
                    with tc.tile_pool(name="ph3", bufs=3) as O, \
                         tc.tile_pool(name="ps3", bufs=2, space="PSUM") as PS3:
                        for sc in range(16):
                            ops = PS3.tile([128, 1024], f32, tag="out",
                                           name="out_ps")
                            for hvc in range(4):
                                for dmc in range(2):
                                    nc.tensor.matmul(
                                        ops[:, dmc * 512:(dmc + 1) * 512],
                                        lhsT=ctxT[:, hvc, sc * 128:(sc + 1) * 128],
                                        rhs=wo_sb[:, hvc, dmc * 512:(dmc + 1) * 512],
                                        start=(hvc == 0), stop=(hvc == 3))
                            o_sb = O.tile([128, 1024], f32, tag="osb")
                            nc.vector.tensor_copy(o_sb[:], ops[:])
                            nc.sync.dma_start(
                                out_d[sc * 128:(sc + 1) * 128, :], o_sb[:])

                    if dbg:
                        nc.sync.dma_start(dbg_qhT[:], qhT[:])
                        nc.sync.dma_start(dbg_khT[:], khT[:])
                        nc.sync.dma_start(dbg_vh[:], vh[:])
                        nc.sync.dma_start(dbg_ctxT[:], ctxT[:])

    nc.compile()
    return nc


def _rope_tables():
    half = D // 2
    inv_freq = (1.0 / (np.float32(ROPE_BASE) **
                       (np.arange(half, dtype=np.float32) / np.float32(half))))
    ang = (np.arange(S, dtype=np.float32)[:, None].astype(np.float32)
           * inv_freq[None, :]).astype(np.float32)
    return (np.cos(ang).astype(np.float16),
            np.sin(ang).astype(np.float16))


def kernel(query, key, value, Wq, bq, Wk, bk, Wv, bv, Wo, bo):
    from concourse.bass_utils import run_bass_kernel_spmd

    if "nc" not in _cached:
        _cached["nc"] = _build_program()
    nc = _cached["nc"]

    cos_t, sin_t = _rope_tables()

    def wlayout(w):  # [1024, 512] -> [128, 8, 512]
        return np.ascontiguousarray(
            w.reshape(8, 128, w.shape[1]).transpose(1, 0, 2)).astype(np.float16)

    in_maps = []
    for c in range(N_CORES):
        b, hg = divmod(c, 2)
        hs = slice(hg * H_LOC, (hg + 1) * H_LOC)
        in_maps.append({
            "xq": np.ascontiguousarray(query[b]).astype(np.float32),
            "xk": np.ascontiguousarray(key[b]).astype(np.float32),
            "xv": np.ascontiguousarray(value[b]).astype(np.float32),
            "wq": wlayout(np.asarray(Wq)[:, hs, :].reshape(D, HK)),
            "wk": wlayout(np.asarray(Wk)[:, hs, :].reshape(D, HK)),
            "wv": wlayout(np.asarray(Wv)[:, hs, :].reshape(D, HK)),
            "wo": np.ascontiguousarray(
                np.asarray(Wo)[hs].reshape(HK, D).reshape(4, 128, D)
                .transpose(1, 0, 2)).astype(np.float16),
            "cost": cos_t,
            "sint": sin_t,
            "ident": np.eye(128, dtype=np.float16),
        })

    _cached["in_maps"] = in_maps
    res = run_bass_kernel_spmd(nc, in_maps, core_ids=list(range(N_CORES)))
    outs = [r["out"] for r in res.results]
    full = np.stack([outs[2 * b] + outs[2 * b + 1] for b in range(4)])
    full = full + np.asarray(bo, dtype=np.float32)[None, None, :]
    return full.astype(np.float32)


# revision 23
# speedup vs baseline: 1.1057x; 1.1057x over previous
"""Trainium2 Bass kernel for a cached-attention block (LN + RoPE + MHA).

Sharding over 8 cores: data-parallel over batch (4) x tensor-parallel over
heads (16 -> 2 groups of 8). Core c handles batch c//2, head-group c%2.
Each core computes a partial output projection (its 8 heads); the two
partials per batch are summed on the host (the all-reduce of the hint).

Per-core dataflow (S=2048, D=1024, 8 local heads, head dim 64):
  Phase 1 (one merged software-pipelined loop over 16 s-chunks):
    LN stats via bn_stats (DVE) + sqrt (ACT) + reciprocal (DVE);
    normalize on ACT (Identity, scale=rstd, bias=-mu*rstd from Pool);
    RoPE on q/k split between DVE (first half) and Pool (second half);
    PE transpose to [d, s]; projections on PE -> qhT/khT [hk, s] fp16 and
    vh [sk, hv] fp16 with a leading all-ones column per head.
  Phase 2 (attention, per q-half then head-pair):
    scores^T [sk, q] fp16 PSUM = khT.T @ qhT (K=64, two heads row-packed
    via tile_position auto-derive); exp on ACT (the phase bottleneck);
    ctx^T [65, q] fp32 accumulated over sk chunks, row 0 = prob sums;
    reciprocal of sums (fast DVE op, directly from PSUM), DMA broadcast,
    normalize during evacuation (DVE).
    After each q-half's 4 head-pairs: output projection for those 8
    s-chunks on PE, DMA straight from PSUM -> overlaps the other
    q-half's ACT-bound attention.
"""

import numpy as np

S = 2048
D = 1024
H_LOC = 8  # heads per core
HK = H_LOC * 64  # 512
N_CORES = 8
EPS = 1e-6
ROPE_BASE = 10000.0

_cached = {}


def _build_program(dbg=False, loop_k=None, ph=(1, 1, 1, 1)):
    import contextlib

    import concourse.tile as tile
    from concourse import bacc, mybir

    f32 = mybir.dt.float32
    f16 = mybir.dt.float16
    AF = mybir.ActivationFunctionType
    OP = mybir.AluOpType

    nc = bacc.Bacc("TRN2", target_bir_lowering=False, debug=False,
                   num_devices=N_CORES)

    xq_d = nc.dram_tensor("xq", [S, D], f32, kind="ExternalInput").ap()
    xk_d = nc.dram_tensor("xk", [S, D], f32, kind="ExternalInput").ap()
    xv_d = nc.dram_tensor("xv", [S, D], f32, kind="ExternalInput").ap()
    wq_d = nc.dram_tensor("wq", [128, 8, HK], f16, kind="ExternalInput").ap()
    wk_d = nc.dram_tensor("wk", [128, 8, HK], f16, kind="ExternalInput").ap()
    wv_d = nc.dram_tensor("wv", [128, 8, HK], f16, kind="ExternalInput").ap()
    wo_d = nc.dram_tensor("wo", [128, 4, D], f16, kind="ExternalInput").ap()
    cos_d = nc.dram_tensor("cost", [S, D // 2], f16, kind="ExternalInput").ap()
    sin_d = nc.dram_tensor("sint", [S, D // 2], f16, kind="ExternalInput").ap()
    id_d = nc.dram_tensor("ident", [128, 128], f16, kind="ExternalInput").ap()
    out_d = nc.dram_tensor("out", [S, D], f32, kind="ExternalOutput").ap()
    if dbg:
        dbg_qhT = nc.dram_tensor("dbg_qhT", [128, 4, S], f16,
                                 kind="ExternalOutput").ap()
        dbg_khT = nc.dram_tensor("dbg_khT", [128, 4, S], f16,
                                 kind="ExternalOutput").ap()
        dbg_vh = nc.dram_tensor("dbg_vh", [128, 16, H_LOC * 65], f16,
                                kind="ExternalOutput").ap()
        dbg_ctxT = nc.dram_tensor("dbg_ctxT", [128, 4, S], f16,
                                  kind="ExternalOutput").ap()
        dbg_pr = nc.dram_tensor("dbg_pr", [128, 16, 1024], f16,
                                kind="ExternalOutput").ap()
        dbg_qT = nc.dram_tensor("dbg_qT", [128, 8, 512], f16,
                                kind="ExternalOutput").ap()
        dbg_ctxu = nc.dram_tensor("dbg_ctxu", [65, 1024], f32,
                                  kind="ExternalOutput").ap()
        dbg_rs = nc.dram_tensor("dbg_rs", [1, 1024], f32,
                                kind="ExternalOutput").ap()

    with tile.TileContext(nc) as tc:
        with tc.tile_pool(name="persist", bufs=1) as P:
            # --- persistent SBUF ---
            wq_sb = P.tile([128, 8, HK], f16, tag="wq")
            wk_sb = P.tile([128, 8, HK], f16, tag="wk")
            wv_sb = P.tile([128, 8, HK], f16, tag="wv")
            wo_sb = P.tile([128, 4, D], f16, tag="wo")
            nc.sync.dma_start(wq_sb[:], wq_d[:])
            nc.sync.dma_start(wk_sb[:], wk_d[:])
            nc.sync.dma_start(wv_sb[:], wv_d[:])
            nc.sync.dma_start(wo_sb[:], wo_d[:])

            qhT = P.tile([128, 4, S], f16, tag="qhT")   # [hk, s] head h: part 64*(h%2), chunk h//2
            khT = P.tile([128, 4, S], f16, tag="khT")
            vh = P.tile([128, 16, H_LOC * 65], f16, tag="vh")  # [sk, skc, h*65+v]; col h*65+0 = 1
            ctxT = P.tile([128, 4, S], f16, tag="ctxT")
            eps_sb = P.tile([128, 1], f32, tag="eps")
            nc.vector.memset(eps_sb[:], EPS)
            id_sb = P.tile([128, 128], f16, tag="ident")
            nc.sync.dma_start(id_sb[:], id_d[:])
            vh3 = vh[:].rearrange("p c (h u) -> p c h u", u=65)
            for h in range(H_LOC):
                nc.vector.memset(vh3[:, :, h, 64:65], 1.0)

            if not ph[0]:
                # phase-isolated benches: init tensors phase 1 would write
                nc.vector.memset(qhT[:], 0.01)
                nc.vector.memset(khT[:], 0.01)
                nc.vector.memset(ctxT[:], 0.01)

            loop_cm = (tc.For_i(0, loop_k, 1) if loop_k is not None
                       else contextlib.nullcontext())

            with loop_cm, tc.tile_pool(name="slab", bufs=2) as SL:
                slabs = {}
                if ph[0]:
                    # ---------------- Phase 1: merged q/k/v pipeline --------
                    with tc.tile_pool(name="ph1x", bufs=2) as X, \
                         tc.tile_pool(name="ph1", bufs=3) as Q, \
                         tc.tile_pool(name="ps1x", bufs=2, space="PSUM") as PSX, \
                         tc.tile_pool(name="ps1p", bufs=2, space="PSUM") as PSP:

                        def ln_stats(x_t, tag):
                            stats = Q.tile([128, 2, 6], f32, tag=f"st_{tag}",
                                           name="st")
                            mv = Q.tile([128, 2], f32, tag=f"mv_{tag}",
                                        name="mv")
                            sq = Q.tile([128, 1], f32, tag=f"sq_{tag}",
                                        name="sq")
                            rstd = Q.tile([128, 1], f32, tag=f"rs_{tag}",
                                          name="rstd")
                            nmr = Q.tile([128, 1], f32, tag=f"nm_{tag}",
                                         name="nmr")
                            nc.vector.bn_stats(stats[:, 0, :], x_t[:, 0:512])
                            nc.vector.bn_stats(stats[:, 1, :], x_t[:, 512:1024])
                            nc.vector.bn_aggr(mv[:], stats[:])
                            nc.scalar.activation(sq[:], mv[:, 1:2], AF.Sqrt,
                                                 bias=eps_sb[:])
                            nc.vector.reciprocal(rstd[:], sq[:])
                            nc.gpsimd.tensor_scalar(
                                nmr[:], mv[:, 0:1], rstd[:], -1.0,
                                OP.mult, OP.mult)
                            return rstd, nmr

                        def normalize(x_t, out_t, rstd, nmr):
                            nc.scalar.activation(out_t[:], x_t[:], AF.Identity,
                                                 bias=nmr[:], scale=rstd[:])

                        def project(sg):
                            qTg, kTg = slabs.pop(sg)
                            if dbg and sg == 0:
                                nc.sync.dma_start(dbg_qT[:], qTg[:])
                            for hkc in range(4):
                                for w_sb, src, dst in ((wq_sb, qTg, qhT),
                                                       (wk_sb, kTg, khT)):
                                    ps = PSP.tile([128, 512], f32, tag="pj",
                                                  name="pj")
                                    for dc in range(8):
                                        nc.tensor.matmul(
                                            ps[:],
                                            lhsT=w_sb[:, dc, hkc * 128:(hkc + 1) * 128],
                                            rhs=src[:, dc, :],
                                            start=(dc == 0), stop=(dc == 7))
                                    nc.scalar.copy(
                                        dst[:, hkc, sg * 512:(sg + 1) * 512],
                                        ps[:])

                        prev = None
                        for i in range(17):
                            cur = None
                            if i < 16:
                                if i % 4 == 0:
                                    slabs[i // 4] = (
                                        SL.tile([128, 8, 512], f16, tag="qTg",
                                                name="qTg"),
                                        SL.tile([128, 8, 512], f16, tag="kTg",
                                                name="kTg"))
                                cos_t = Q.tile([128, 512], f16, tag="cos")
                                sin_t = Q.tile([128, 512], f16, tag="sin")
                                nc.sync.dma_start(cos_t[:], cos_d[i * 128:(i + 1) * 128, :])
                                nc.sync.dma_start(sin_t[:], sin_d[i * 128:(i + 1) * 128, :])
                                cur = {"cos": cos_t, "sin": sin_t, "i": i}
                                for nm, x_d in (("q", xq_d), ("k", xk_d),
                                                ("v", xv_d)):
                                    x_t = X.tile([128, D], f32, tag=f"x{nm}",
                                                 name="x_t")
                                    nc.sync.dma_start(x_t[:], x_d[i * 128:(i + 1) * 128, :])
                                    rstd, nmr = ln_stats(x_t, nm)
                                    cur[nm] = (x_t, rstd, nmr)
                            if prev is not None:
                                pi = prev["i"]
                                psg, pst = pi // 4, pi % 4
                                # ---- v path ----
                                x_t, rstd, nmr = prev["v"]
                                vn_t = X.tile([128, D], f16, tag="nv",
                                              name="vn_t")
                                normalize(x_t, vn_t, rstd, nmr)
                                xp_ps = PSX.tile([128, 8, 128], f16, tag="xpv",
                                                 name="xp_v")
                                for dc in range(8):
                                    nc.tensor.transpose(
                                        xp_ps[:, dc, :],
                                        vn_t[:, dc * 128:(dc + 1) * 128],
                                        id_sb[:])
                                vT_t = X.tile([128, 8, 128], f16, tag="vT",
                                              name="vT_t")
                                nc.vector.tensor_copy(vT_t[:], xp_ps[:])
                                vp = PSP.tile([128, HK], f32, tag="pj",
                                              name="vp")
                                for dc in range(8):
                                    nc.tensor.matmul(vp[:], lhsT=vT_t[:, dc, :],
                                                     rhs=wv_sb[:, dc, :],
                                                     start=(dc == 0),
                                                     stop=(dc == 7))
                                nc.scalar.copy(
                                    vh3[:, pi, :, 0:64],
                                    vp[:].rearrange("p (h v) -> p h v", v=64))
                                # ---- q/k path ----
                                xps = {}
                                for nm in ("q", "k"):
                                    x_t, rstd, nmr = prev[nm]
                                    n_t = Q.tile([128, D], f16, tag=f"n{nm}",
                                                 name="n_t")
                                    normalize(x_t, n_t, rstd, nmr)
                                    r_t = Q.tile([128, D], f16, tag=f"r{nm}",
                                                 name="r_t")
                                    b = Q.tile([128, 512], f16, tag=f"b{nm}",
                                               name="b")
                                    d = Q.tile([128, 512], f16, tag=f"d{nm}",
                                               name="d")
                                    # first rope half on DVE (in-place sub)
                                    nc.vector.tensor_mul(
                                        r_t[:, 0:512], n_t[:, 0:512],
                                        prev["cos"][:])
                                    nc.vector.tensor_mul(
                                        b[:], n_t[:, 512:1024], prev["sin"][:])
                                    nc.vector.tensor_sub(
                                        r_t[:, 0:512], r_t[:, 0:512], b[:])
                                    # second rope half: muls on Pool,
                                    # final add on DVE
                                    nc.gpsimd.tensor_mul(
                                        r_t[:, 512:1024], n_t[:, 512:1024],
                                        prev["cos"][:])
                                    nc.gpsimd.tensor_mul(
                                        d[:], n_t[:, 0:512], prev["sin"][:])
                                    nc.vector.tensor_add(
                                        r_t[:, 512:1024], r_t[:, 512:1024],
                                        d[:])
                                    xq_ps = PSX.tile([128, 8, 128], f16,
                                                     tag=f"xp{nm}", name="xq_ps")
                                    for dc in range(8):
                                        nc.tensor.transpose(
                                            xq_ps[:, dc, :],
                                            r_t[:, dc * 128:(dc + 1) * 128],
                                            id_sb[:])
                                    xps[nm] = xq_ps
                                for nm, dst_i in (("q", 0), ("k", 1)):
                                    nc.scalar.copy(
                                        slabs[psg][dst_i][:, :, pst * 128:(pst + 1) * 128],
                                        xps[nm][:])
                                if pst == 3 and psg < 3:
                                    # sg3's projections are deferred into
                                    # phase 2 (hidden under ACT-bound passes)
                                    project(psg)
                            prev = cur

                if ph[2]:
                    # ---------------- Phase 2: attention + out-proj ---------
                    import concourse.bass as bass
                    with tc.tile_pool(name="ph2", bufs=10) as PR, \
                         tc.tile_pool(name="ph2b", bufs=2) as RS, \
                         tc.tile_pool(name="ps2", bufs=1, space="PSUM") as PS2:
                        for qc in range(2):
                            q0 = qc * 1024
                            for hp in range(4):
                                h0, h1 = 2 * hp, 2 * hp + 1
                                ctx_ps = None
                                # software pipeline: ctx matmuls for chunk
                                # skc-2 are issued after the scores/exp of
                                # chunk skc, so the PE never sits directly
                                # behind the exp it is waiting for.
                                hist = {}
                                for skc in range(18):
                                    cur = [None, None]
                                    if skc < 16:
                                        st_ps = [PS2.tile([128, 1024], f32, tag=f"sc{e}",
                                                           name=f"sc{e}") for e in range(2)]
                                        for j in range(2):
                                            for e, h in ((0, h0), (1, h1)):
                                                pb = slice(64 * (h % 2), 64 * (h % 2) + 64)
                                                nc.tensor.matmul(
                                                    st_ps[e][:, j * 512:(j + 1) * 512],
                                                    lhsT=khT[pb, hp, skc * 128:(skc + 1) * 128],
                                                    rhs=qhT[pb, hp, q0 + j * 512:q0 + (j + 1) * 512])
                                        for e, h in ((0, h0), (1, h1)):
                                            pr = PR.tile([128, 1024], f16, tag=f"pr{e}")
                                            nc.scalar.activation(pr[:], st_ps[e][:], AF.Exp,
                                                                 scale=0.125)
                                            if dbg and hp == 0 and qc == 0 and e == 0:
                                                nc.sync.dma_start(dbg_pr[:, skc, :], pr[:])
                                            cur[e] = pr
                                        hist[skc] = cur
                                    if skc == 1 and qc == 0 and ph[0]:
                                        # deferred sg3 projection for this
                                        # head-pair (hkc == hp): runs on PE
                                        # under the first exps; scratch in
                                        # the ctx tags (freed by the DVE
                                        # evac before the first ctx matmul
                                        # at skc == 2), evac on DVE
                                        qTg3, kTg3 = slabs[3]
                                        for ee, (w_sb, src3, dst) in enumerate(
                                                ((wq_sb, qTg3, qhT),
                                                 (wk_sb, kTg3, khT))):
                                            pps = PS2.tile([128, 1024], f32,
                                                           tag=f"ctx{ee}",
                                                           name="pj3")
                                            for dc in range(8):
                                                nc.tensor.matmul(
                                                    pps[:, 0:512],
                                                    lhsT=w_sb[:, dc, hp * 128:(hp + 1) * 128],
                                                    rhs=src3[:, dc, :],
                                                    start=(dc == 0),
                                                    stop=(dc == 7))
                                            nc.vector.tensor_copy(
                                                dst[:, hp, 3 * 512:4 * 512],
                                                pps[:, 0:512])
                                    if skc == 2:
                                        ctx_ps = [PS2.tile([65, 1024], f32,
                                                            tag=f"ctx{e}",
                                                            name=f"ctx{e}")
                                                  for e in range(2)]
                                    pskc = skc - 2
                                    if pskc >= 0:
                                        pcur = hist.pop(pskc)
                                        for e, h in ((0, h0), (1, h1)):
                                            lw = vh3[:, pskc, h, 0:65]
                                            for j in range(2):
                                                nc.tensor.matmul(
                                                    ctx_ps[e][:, j * 512:(j + 1) * 512],
                                                    lhsT=lw,
                                                    rhs=pcur[e][:, j * 512:(j + 1) * 512],
                                                    start=(pskc == 0), stop=(pskc == 15))
                                if dbg and hp == 0 and qc == 0:
                                    ctxu_sb = RS.tile([65, 1024], f32, tag="dbgu")
                                    nc.vector.tensor_copy(ctxu_sb[:], ctx_ps[0][:])
                                    nc.sync.dma_start(dbg_ctxu[:], ctxu_sb[:])
                                for e, h in ((0, h0), (1, h1)):
                                    # sums row (partition 64) staged to
                                    # partition 0 for the custom DVE recip;
                                    # the normalize mul reads ctx PSUM
                                    # directly during evacuation.
                                    s_sb = RS.tile([1, 1024], f32, tag="ssum")
                                    nc.vector.tensor_copy(s_sb[:], ctx_ps[e][64:65, :])
                                    cu = RS.tile([64, 1024], f16, tag="cu")
                                    nc.vector.tensor_copy(cu[:], ctx_ps[e][0:64, :])
                                    rs = RS.tile([1, 1024], f32, tag="rs")
                                    nc.vector.reciprocal_approx_fast(
                                        out=rs[:], in_=s_sb[:])
                                    rs16 = RS.tile([1, 1024], f16, tag="rs16")
                                    nc.vector.tensor_copy(rs16[:], rs[:])
                                    rsb = RS.tile([64, 1024], f16, tag="rsb")
                                    nc.gpsimd.partition_broadcast(
                                        rsb[:], rs16[:], channels=64)
                                    if dbg and hp == 0 and qc == 0 and e == 0:
                                        nc.sync.dma_start(dbg_rs[:], rs[:])
                                    nc.vector.tensor_mul(
                                        ctxT[64 * (h % 2):64 * (h % 2) + 64, h // 2,
                                             q0:q0 + 1024],
                                        cu[:], rsb[:])
                    # ---------------- Phase 3: output projection ----------
                    with tc.tile_pool(name="ph3", bufs=3) as O, \
                         tc.tile_pool(name="ps3", bufs=4, space="PSUM") as PS3:
                        for sc in range(16):
                            ops = PS3.tile([128, 1024], f32, tag="out",
                                           name="out_ps")
                            for hvc in range(4):
                                for dmc in range(2):
                                    nc.tensor.matmul(
                                        ops[:, dmc * 512:(dmc + 1) * 512],
                                        lhsT=ctxT[:, hvc, sc * 128:(sc + 1) * 128],
                                        rhs=wo_sb[:, hvc, dmc * 512:(dmc + 1) * 512],
                                        start=(hvc == 0), stop=(hvc == 3))
                            o_sb = O.tile([128, 1024], f32, tag="osb")
                            nc.vector.tensor_copy(o_sb[:], ops[:])
                            nc.sync.dma_start(
                                out_d[sc * 128:(sc + 1) * 128, :], o_sb[:])

                    if dbg:
                        nc.sync.dma_start(dbg_qhT[:], qhT[:])
                        nc.sync.dma_start(dbg_khT[:], khT[:])
                        nc.sync.dma_start(dbg_vh[:], vh[:])
                        nc.sync.dma_start(dbg_ctxT[:], ctxT[:])

    nc.compile()
    return nc


def _rope_tables():
    half = D // 2
    inv_freq = (1.0 / (np.float32(ROPE_BASE) **
                       (np.arange(half, dtype=np.float32) / np.float32(half))))
    ang = (np.arange(S, dtype=np.float32)[:, None].astype(np.float32)
           * inv_freq[None, :]).astype(np.float32)
    return (np.cos(ang).astype(np.float16),
            np.sin(ang).astype(np.float16))


def kernel(query, key, value, Wq, bq, Wk, bk, Wv, bv, Wo, bo):
    from concourse.bass_utils import run_bass_kernel_spmd

    if "nc" not in _cached:
        _cached["nc"] = _build_program()
    nc = _cached["nc"]

    cos_t, sin_t = _rope_tables()

    def wlayout(w):  # [1024, 512] -> [128, 8, 512]
        return np.ascontiguousarray(
            w.reshape(8, 128, w.shape[1]).transpose(1, 0, 2)).astype(np.float16)

    in_maps = []
    for c in range(N_CORES):
        b, hg = divmod(c, 2)
        hs = slice(hg * H_LOC, (hg + 1) * H_LOC)
        in_maps.append({
            "xq": np.ascontiguousarray(query[b]).astype(np.float32),
            "xk": np.ascontiguousarray(key[b]).astype(np.float32),
            "xv": np.ascontiguousarray(value[b]).astype(np.float32),
            "wq": wlayout(np.asarray(Wq)[:, hs, :].reshape(D, HK)),
            "wk": wlayout(np.asarray(Wk)[:, hs, :].reshape(D, HK)),
            "wv": wlayout(np.asarray(Wv)[:, hs, :].reshape(D, HK)),
            "wo": np.ascontiguousarray(
                np.asarray(Wo)[hs].reshape(HK, D).reshape(4, 128, D)
                .transpose(1, 0, 2)).astype(np.float16),
            "cost": cos_t,
            "sint": sin_t,
            "ident": np.eye(128, dtype=np.float16),
        })

    _cached["in_maps"] = in_maps
    res = run_bass_kernel_spmd(nc, in_maps, core_ids=list(range(N_CORES)))
    outs = [r["out"] for r in res.results]
    full = np.stack([outs[2 * b] + outs[2 * b + 1] for b in range(4)])
    full = full + np.asarray(bo, dtype=np.float32)[None, None, :]
    return full.astype(np.float32)


# revision 24
# speedup vs baseline: 1.1607x; 1.0498x over previous
"""Trainium2 Bass kernel for a cached-attention block (LN + RoPE + MHA).

Sharding over 8 cores: data-parallel over batch (4) x tensor-parallel over
heads (16 -> 2 groups of 8). Core c handles batch c//2, head-group c%2.
Each core computes a partial output projection (its 8 heads); the two
partials per batch are summed on the host (the all-reduce of the hint).

Per-core dataflow (S=2048, D=1024, 8 local heads, head dim 64):
  Phase 1 (one merged software-pipelined loop over 16 s-chunks):
    LN stats via bn_stats (DVE) + sqrt (ACT) + reciprocal (DVE);
    normalize on ACT (Identity, scale=rstd, bias=-mu*rstd from Pool);
    RoPE on q/k split between DVE (first half) and Pool (second half);
    PE transpose to [d, s]; projections on PE -> qhT/khT [hk, s] fp16 and
    vh [sk, hv] fp16 with a leading all-ones column per head.
  Phase 2 (attention, per q-half then head-pair):
    scores^T [sk, q] fp16 PSUM = khT.T @ qhT (K=64, two heads row-packed
    via tile_position auto-derive); exp on ACT (the phase bottleneck);
    ctx^T [65, q] fp32 accumulated over sk chunks, row 0 = prob sums;
    reciprocal of sums (fast DVE op, directly from PSUM), DMA broadcast,
    normalize during evacuation (DVE).
    After each q-half's 4 head-pairs: output projection for those 8
    s-chunks on PE, DMA straight from PSUM -> overlaps the other
    q-half's ACT-bound attention.
"""

import numpy as np

S = 2048
D = 1024
H_LOC = 8  # heads per core
HK = H_LOC * 64  # 512
N_CORES = 8
EPS = 1e-6
ROPE_BASE = 10000.0

_cached = {}


def _build_program(dbg=False, loop_k=None, ph=(1, 1, 1, 1)):
    import contextlib

    import concourse.tile as tile
    from concourse import bacc, mybir

    f32 = mybir.dt.float32
    f16 = mybir.dt.float16
    AF = mybir.ActivationFunctionType
    OP = mybir.AluOpType

    nc = bacc.Bacc("TRN2", target_bir_lowering=False, debug=False,
                   num_devices=N_CORES)

    xq_d = nc.dram_tensor("xq", [S, D], f32, kind="ExternalInput").ap()
    xk_d = nc.dram_tensor("xk", [S, D], f32, kind="ExternalInput").ap()
    xv_d = nc.dram_tensor("xv", [S, D], f32, kind="ExternalInput").ap()
    wq_d = nc.dram_tensor("wq", [128, 8, HK], f16, kind="ExternalInput").ap()
    wk_d = nc.dram_tensor("wk", [128, 8, HK], f16, kind="ExternalInput").ap()
    wv_d = nc.dram_tensor("wv", [128, 8, HK], f16, kind="ExternalInput").ap()
    wo_d = nc.dram_tensor("wo", [128, 4, D], f16, kind="ExternalInput").ap()
    cos_d = nc.dram_tensor("cost", [S, D // 2], f16, kind="ExternalInput").ap()
    sin_d = nc.dram_tensor("sint", [S, D // 2], f16, kind="ExternalInput").ap()
    id_d = nc.dram_tensor("ident", [128, 128], f16, kind="ExternalInput").ap()
    out_d = nc.dram_tensor("out", [S, D], f32, kind="ExternalOutput").ap()
    if dbg:
        dbg_qhT = nc.dram_tensor("dbg_qhT", [128, 4, S], f16,
                                 kind="ExternalOutput").ap()
        dbg_khT = nc.dram_tensor("dbg_khT", [128, 4, S], f16,
                                 kind="ExternalOutput").ap()
        dbg_vh = nc.dram_tensor("dbg_vh", [128, 16, H_LOC * 65], f16,
                                kind="ExternalOutput").ap()
        dbg_ctxT = nc.dram_tensor("dbg_ctxT", [128, 4, S], f16,
                                  kind="ExternalOutput").ap()
        dbg_pr = nc.dram_tensor("dbg_pr", [128, 16, 1024], f16,
                                kind="ExternalOutput").ap()
        dbg_qT = nc.dram_tensor("dbg_qT", [128, 8, 512], f16,
                                kind="ExternalOutput").ap()
        dbg_ctxu = nc.dram_tensor("dbg_ctxu", [65, 1024], f32,
                                  kind="ExternalOutput").ap()
        dbg_rs = nc.dram_tensor("dbg_rs", [1, 1024], f32,
                                kind="ExternalOutput").ap()

    with tile.TileContext(nc) as tc:
        with tc.tile_pool(name="persist", bufs=1) as P:
            # --- persistent SBUF ---
            wq_sb = P.tile([128, 8, HK], f16, tag="wq")
            wk_sb = P.tile([128, 8, HK], f16, tag="wk")
            wv_sb = P.tile([128, 8, HK], f16, tag="wv")
            wo_sb = P.tile([128, 4, D], f16, tag="wo")
            nc.sync.dma_start(wq_sb[:], wq_d[:])
            nc.sync.dma_start(wk_sb[:], wk_d[:])
            nc.sync.dma_start(wv_sb[:], wv_d[:])
            nc.sync.dma_start(wo_sb[:], wo_d[:])

            qhT = P.tile([128, 4, S], f16, tag="qhT")   # [hk, s] head h: part 64*(h%2), chunk h//2
            khT = P.tile([128, 4, S], f16, tag="khT")
            vh = P.tile([128, 16, H_LOC * 65], f16, tag="vh")  # [sk, skc, h*65+v]; col h*65+0 = 1
            ctxT = P.tile([128, 4, S], f16, tag="ctxT")
            eps_sb = P.tile([128, 1], f32, tag="eps")
            nc.vector.memset(eps_sb[:], EPS)
            id_sb = P.tile([128, 128], f16, tag="ident")
            nc.sync.dma_start(id_sb[:], id_d[:])
            vh3 = vh[:].rearrange("p c (h u) -> p c h u", u=65)
            for h in range(H_LOC):
                nc.vector.memset(vh3[:, :, h, 64:65], 1.0)

            if not ph[0]:
                # phase-isolated benches: init tensors phase 1 would write
                nc.vector.memset(qhT[:], 0.01)
                nc.vector.memset(khT[:], 0.01)
                nc.vector.memset(ctxT[:], 0.01)

            loop_cm = (tc.For_i(0, loop_k, 1) if loop_k is not None
                       else contextlib.nullcontext())

            with loop_cm, tc.tile_pool(name="slab", bufs=2) as SL:
                slabs = {}
                if ph[0]:
                    # ---------------- Phase 1: merged q/k/v pipeline --------
                    with tc.tile_pool(name="ph1x", bufs=2) as X, \
                         tc.tile_pool(name="ph1", bufs=3) as Q, \
                         tc.tile_pool(name="ps1x", bufs=2, space="PSUM") as PSX, \
                         tc.tile_pool(name="ps1p", bufs=2, space="PSUM") as PSP:

                        def ln_stats(x_t, tag):
                            stats = Q.tile([128, 2, 6], f32, tag=f"st_{tag}",
                                           name="st")
                            mv = Q.tile([128, 2], f32, tag=f"mv_{tag}",
                                        name="mv")
                            sq = Q.tile([128, 1], f32, tag=f"sq_{tag}",
                                        name="sq")
                            rstd = Q.tile([128, 1], f32, tag=f"rs_{tag}",
                                          name="rstd")
                            nmr = Q.tile([128, 1], f32, tag=f"nm_{tag}",
                                         name="nmr")
                            nc.vector.bn_stats(stats[:, 0, :], x_t[:, 0:512])
                            nc.vector.bn_stats(stats[:, 1, :], x_t[:, 512:1024])
                            nc.vector.bn_aggr(mv[:], stats[:])
                            nc.scalar.activation(sq[:], mv[:, 1:2], AF.Sqrt,
                                                 bias=eps_sb[:])
                            nc.vector.reciprocal(rstd[:], sq[:])
                            nc.gpsimd.tensor_scalar(
                                nmr[:], mv[:, 0:1], rstd[:], -1.0,
                                OP.mult, OP.mult)
                            return rstd, nmr

                        def normalize(x_t, out_t, rstd, nmr):
                            nc.scalar.activation(out_t[:], x_t[:], AF.Identity,
                                                 bias=nmr[:], scale=rstd[:])

                        def project(sg):
                            qTg, kTg = slabs.pop(sg)
                            if dbg and sg == 0:
                                nc.sync.dma_start(dbg_qT[:], qTg[:])
                            for hkc in range(4):
                                for w_sb, src, dst in ((wq_sb, qTg, qhT),
                                                       (wk_sb, kTg, khT)):
                                    ps = PSP.tile([128, 512], f32, tag="pj",
                                                  name="pj")
                                    for dc in range(8):
                                        nc.tensor.matmul(
                                            ps[:],
                                            lhsT=w_sb[:, dc, hkc * 128:(hkc + 1) * 128],
                                            rhs=src[:, dc, :],
                                            start=(dc == 0), stop=(dc == 7))
                                    nc.scalar.copy(
                                        dst[:, hkc, sg * 512:(sg + 1) * 512],
                                        ps[:])

                        prev = None
                        for i in range(17):
                            cur = None
                            if i < 16:
                                if i % 4 == 0:
                                    slabs[i // 4] = (
                                        SL.tile([128, 8, 512], f16, tag="qTg",
                                                name="qTg"),
                                        SL.tile([128, 8, 512], f16, tag="kTg",
                                                name="kTg"))
                                cos_t = Q.tile([128, 512], f16, tag="cos")
                                sin_t = Q.tile([128, 512], f16, tag="sin")
                                nc.sync.dma_start(cos_t[:], cos_d[i * 128:(i + 1) * 128, :])
                                nc.sync.dma_start(sin_t[:], sin_d[i * 128:(i + 1) * 128, :])
                                cur = {"cos": cos_t, "sin": sin_t, "i": i}
                                for nm, x_d in (("q", xq_d), ("k", xk_d),
                                                ("v", xv_d)):
                                    x_t = X.tile([128, D], f32, tag=f"x{nm}",
                                                 name="x_t")
                                    nc.sync.dma_start(x_t[:], x_d[i * 128:(i + 1) * 128, :])
                                    rstd, nmr = ln_stats(x_t, nm)
                                    cur[nm] = (x_t, rstd, nmr)
                            if prev is not None:
                                pi = prev["i"]
                                psg, pst = pi // 4, pi % 4
                                # ---- v path ----
                                x_t, rstd, nmr = prev["v"]
                                vn_t = X.tile([128, D], f16, tag="nv",
                                              name="vn_t")
                                normalize(x_t, vn_t, rstd, nmr)
                                xp_ps = PSX.tile([128, 8, 128], f16, tag="xpv",
                                                 name="xp_v")
                                for dc in range(8):
                                    nc.tensor.transpose(
                                        xp_ps[:, dc, :],
                                        vn_t[:, dc * 128:(dc + 1) * 128],
                                        id_sb[:])
                                vT_t = X.tile([128, 8, 128], f16, tag="vT",
                                              name="vT_t")
                                nc.vector.tensor_copy(vT_t[:], xp_ps[:])
                                vp = PSP.tile([128, HK], f32, tag="pj",
                                              name="vp")
                                for dc in range(8):
                                    nc.tensor.matmul(vp[:], lhsT=vT_t[:, dc, :],
                                                     rhs=wv_sb[:, dc, :],
                                                     start=(dc == 0),
                                                     stop=(dc == 7))
                                nc.scalar.copy(
                                    vh3[:, pi, :, 0:64],
                                    vp[:].rearrange("p (h v) -> p h v", v=64))
                                # ---- q/k path ----
                                xps = {}
                                for nm in ("q", "k"):
                                    x_t, rstd, nmr = prev[nm]
                                    n_t = Q.tile([128, D], f16, tag=f"n{nm}",
                                                 name="n_t")
                                    normalize(x_t, n_t, rstd, nmr)
                                    r_t = Q.tile([128, D], f16, tag=f"r{nm}",
                                                 name="r_t")
                                    b = Q.tile([128, 512], f16, tag=f"b{nm}",
                                               name="b")
                                    d = Q.tile([128, 512], f16, tag=f"d{nm}",
                                               name="d")
                                    # first rope half on DVE (in-place sub)
                                    nc.vector.tensor_mul(
                                        r_t[:, 0:512], n_t[:, 0:512],
                                        prev["cos"][:])
                                    nc.vector.tensor_mul(
                                        b[:], n_t[:, 512:1024], prev["sin"][:])
                                    nc.vector.tensor_sub(
                                        r_t[:, 0:512], r_t[:, 0:512], b[:])
                                    # second rope half: muls on Pool,
                                    # final add on DVE
                                    nc.gpsimd.tensor_mul(
                                        r_t[:, 512:1024], n_t[:, 512:1024],
                                        prev["cos"][:])
                                    nc.gpsimd.tensor_mul(
                                        d[:], n_t[:, 0:512], prev["sin"][:])
                                    nc.vector.tensor_add(
                                        r_t[:, 512:1024], r_t[:, 512:1024],
                                        d[:])
                                    xq_ps = PSX.tile([128, 8, 128], f16,
                                                     tag=f"xp{nm}", name="xq_ps")
                                    for dc in range(8):
                                        nc.tensor.transpose(
                                            xq_ps[:, dc, :],
                                            r_t[:, dc * 128:(dc + 1) * 128],
                                            id_sb[:])
                                    xps[nm] = xq_ps
                                for nm, dst_i in (("q", 0), ("k", 1)):
                                    nc.scalar.copy(
                                        slabs[psg][dst_i][:, :, pst * 128:(pst + 1) * 128],
                                        xps[nm][:])
                                if pst == 3 and psg < 3:
                                    # sg3's projections are deferred into
                                    # phase 2 (hidden under ACT-bound passes)
                                    project(psg)
                            prev = cur

                if ph[2]:
                    # ---------------- Phase 2: attention + out-proj ---------
                    import concourse.bass as bass
                    with tc.tile_pool(name="ph2", bufs=10) as PR, \
                         tc.tile_pool(name="ph2b", bufs=2) as RS, \
                         tc.tile_pool(name="ps2", bufs=1, space="PSUM") as PS2:
                        for qc in range(2):
                            q0 = qc * 1024
                            for hp in range(4):
                                h0, h1 = 2 * hp, 2 * hp + 1
                                ctx_ps = None
                                # software pipeline: ctx matmuls for chunk
                                # skc-2 are issued after the scores/exp of
                                # chunk skc, so the PE never sits directly
                                # behind the exp it is waiting for.
                                hist = {}
                                for skc in range(18):
                                    cur = [None, None]
                                    if skc < 16:
                                        st_ps = [PS2.tile([128, 1024], f32, tag=f"sc{e}",
                                                           name=f"sc{e}") for e in range(2)]
                                        for j in range(2):
                                            for e, h in ((0, h0), (1, h1)):
                                                pb = slice(64 * (h % 2), 64 * (h % 2) + 64)
                                                nc.tensor.matmul(
                                                    st_ps[e][:, j * 512:(j + 1) * 512],
                                                    lhsT=khT[pb, hp, skc * 128:(skc + 1) * 128],
                                                    rhs=qhT[pb, hp, q0 + j * 512:q0 + (j + 1) * 512])
                                        for e, h in ((0, h0), (1, h1)):
                                            pr = PR.tile([128, 1024], f16, tag=f"pr{e}")
                                            nc.scalar.activation(pr[:], st_ps[e][:], AF.Exp,
                                                                 scale=0.125)
                                            if dbg and hp == 0 and qc == 0 and e == 0:
                                                nc.sync.dma_start(dbg_pr[:, skc, :], pr[:])
                                            cur[e] = pr
                                        hist[skc] = cur
                                    if skc == 1 and qc == 0 and ph[0]:
                                        # deferred sg3 projection for this
                                        # head-pair (hkc == hp): runs on PE
                                        # under the first exps; scratch in
                                        # the ctx tags (freed by the DVE
                                        # evac before the first ctx matmul
                                        # at skc == 2), evac on DVE
                                        qTg3, kTg3 = slabs[3]
                                        for ee, (w_sb, src3, dst) in enumerate(
                                                ((wq_sb, qTg3, qhT),
                                                 (wk_sb, kTg3, khT))):
                                            pps = PS2.tile([128, 1024], f32,
                                                           tag=f"ctx{ee}",
                                                           name="pj3")
                                            for dc in range(8):
                                                nc.tensor.matmul(
                                                    pps[:, 0:512],
                                                    lhsT=w_sb[:, dc, hp * 128:(hp + 1) * 128],
                                                    rhs=src3[:, dc, :],
                                                    start=(dc == 0),
                                                    stop=(dc == 7))
                                            nc.vector.tensor_copy(
                                                dst[:, hp, 3 * 512:4 * 512],
                                                pps[:, 0:512])
                                    if skc == 1 and qc == 1 and ph[3]:
                                        # output projection for two of
                                        # qc0's s-chunks, hidden under the
                                        # first exps of this pass (ctx-tag
                                        # scratch, freed before skc == 2)
                                        for ee in range(2):
                                            sc_o = hp * 2 + ee
                                            pps = PS2.tile([128, 1024], f32,
                                                           tag=f"ctx{ee}",
                                                           name="oproj")
                                            for hvc in range(4):
                                                for dmc in range(2):
                                                    nc.tensor.matmul(
                                                        pps[:, dmc * 512:(dmc + 1) * 512],
                                                        lhsT=ctxT[:, hvc, sc_o * 128:(sc_o + 1) * 128],
                                                        rhs=wo_sb[:, hvc, dmc * 512:(dmc + 1) * 512],
                                                        start=(hvc == 0),
                                                        stop=(hvc == 3))
                                            o_sb = RS.tile([128, 1024], f32,
                                                           tag="osb2")
                                            nc.vector.tensor_copy(o_sb[:], pps[:])
                                            nc.sync.dma_start(
                                                out_d[sc_o * 128:(sc_o + 1) * 128, :],
                                                o_sb[:])
                                    if skc == 2:
                                        ctx_ps = [PS2.tile([65, 1024], f32,
                                                            tag=f"ctx{e}",
                                                            name=f"ctx{e}")
                                                  for e in range(2)]
                                    pskc = skc - 2
                                    if pskc >= 0:
                                        pcur = hist.pop(pskc)
                                        for e, h in ((0, h0), (1, h1)):
                                            lw = vh3[:, pskc, h, 0:65]
                                            for j in range(2):
                                                nc.tensor.matmul(
                                                    ctx_ps[e][:, j * 512:(j + 1) * 512],
                                                    lhsT=lw,
                                                    rhs=pcur[e][:, j * 512:(j + 1) * 512],
                                                    start=(pskc == 0), stop=(pskc == 15))
                                if dbg and hp == 0 and qc == 0:
                                    ctxu_sb = RS.tile([65, 1024], f32, tag="dbgu")
                                    nc.vector.tensor_copy(ctxu_sb[:], ctx_ps[0][:])
                                    nc.sync.dma_start(dbg_ctxu[:], ctxu_sb[:])
                                for e, h in ((0, h0), (1, h1)):
                                    # sums row (partition 64) staged to
                                    # partition 0 for the custom DVE recip;
                                    # the normalize mul reads ctx PSUM
                                    # directly during evacuation.
                                    s_sb = RS.tile([1, 1024], f32, tag="ssum")
                                    nc.vector.tensor_copy(s_sb[:], ctx_ps[e][64:65, :])
                                    cu = RS.tile([64, 1024], f16, tag="cu")
                                    nc.vector.tensor_copy(cu[:], ctx_ps[e][0:64, :])
                                    rs = RS.tile([1, 1024], f32, tag="rs")
                                    nc.vector.reciprocal_approx_fast(
                                        out=rs[:], in_=s_sb[:])
                                    rs16 = RS.tile([1, 1024], f16, tag="rs16")
                                    nc.vector.tensor_copy(rs16[:], rs[:])
                                    rsb = RS.tile([64, 1024], f16, tag="rsb")
                                    nc.gpsimd.partition_broadcast(
                                        rsb[:], rs16[:], channels=64)
                                    if dbg and hp == 0 and qc == 0 and e == 0:
                                        nc.sync.dma_start(dbg_rs[:], rs[:])
                                    nc.vector.tensor_mul(
                                        ctxT[64 * (h % 2):64 * (h % 2) + 64, h // 2,
                                             q0:q0 + 1024],
                                        cu[:], rsb[:])
                    # ---------------- Phase 3: output projection ----------
                    with tc.tile_pool(name="ph3", bufs=3) as O, \
                         tc.tile_pool(name="ps3", bufs=2, space="PSUM") as PS3:
                        for sc in range(8 if ph[2] else 0, 16):
                            ops = PS3.tile([128, 1024], f32, tag="out",
                                           name="out_ps")
                            for hvc in range(4):
                                for dmc in range(2):
                                    nc.tensor.matmul(
                                        ops[:, dmc * 512:(dmc + 1) * 512],
                                        lhsT=ctxT[:, hvc, sc * 128:(sc + 1) * 128],
                                        rhs=wo_sb[:, hvc, dmc * 512:(dmc + 1) * 512],
                                        start=(hvc == 0), stop=(hvc == 3))
                            o_sb = O.tile([128, 1024], f32, tag="osb")
                            nc.vector.tensor_copy(o_sb[:], ops[:])
                            nc.sync.dma_start(
                                out_d[sc * 128:(sc + 1) * 128, :], o_sb[:])

                    if dbg:
                        nc.sync.dma_start(dbg_qhT[:], qhT[:])
                        nc.sync.dma_start(dbg_khT[:], khT[:])
                        nc.sync.dma_start(dbg_vh[:], vh[:])
                        nc.sync.dma_start(dbg_ctxT[:], ctxT[:])

    nc.compile()
    return nc


def _rope_tables():
    half = D // 2
    inv_freq = (1.0 / (np.float32(ROPE_BASE) **
                       (np.arange(half, dtype=np.float32) / np.float32(half))))
    ang = (np.arange(S, dtype=np.float32)[:, None].astype(np.float32)
           * inv_freq[None, :]).astype(np.float32)
    return (np.cos(ang).astype(np.float16),
            np.sin(ang).astype(np.float16))


def kernel(query, key, value, Wq, bq, Wk, bk, Wv, bv, Wo, bo):
    from concourse.bass_utils import run_bass_kernel_spmd

    if "nc" not in _cached:
        _cached["nc"] = _build_program()
    nc = _cached["nc"]

    cos_t, sin_t = _rope_tables()

    def wlayout(w):  # [1024, 512] -> [128, 8, 512]
        return np.ascontiguousarray(
            w.reshape(8, 128, w.shape[1]).transpose(1, 0, 2)).astype(np.float16)

    in_maps = []
    for c in range(N_CORES):
        b, hg = divmod(c, 2)
        hs = slice(hg * H_LOC, (hg + 1) * H_LOC)
        in_maps.append({
            "xq": np.ascontiguousarray(query[b]).astype(np.float32),
            "xk": np.ascontiguousarray(key[b]).astype(np.float32),
            "xv": np.ascontiguousarray(value[b]).astype(np.float32),
            "wq": wlayout(np.asarray(Wq)[:, hs, :].reshape(D, HK)),
            "wk": wlayout(np.asarray(Wk)[:, hs, :].reshape(D, HK)),
            "wv": wlayout(np.asarray(Wv)[:, hs, :].reshape(D, HK)),
            "wo": np.ascontiguousarray(
                np.asarray(Wo)[hs].reshape(HK, D).reshape(4, 128, D)
                .transpose(1, 0, 2)).astype(np.float16),
            "cost": cos_t,
            "sint": sin_t,
            "ident": np.eye(128, dtype=np.float16),
        })

    _cached["in_maps"] = in_maps
    res = run_bass_kernel_spmd(nc, in_maps, core_ids=list(range(N_CORES)))
    outs = [r["out"] for r in res.results]
    full = np.stack([outs[2 * b] + outs[2 * b + 1] for b in range(4)])
    full = full + np.asarray(bo, dtype=np.float32)[None, None, :]
    return full.astype(np.float32)


# revision 25
# speedup vs baseline: 1.1995x; 1.0334x over previous
"""Trainium2 Bass kernel for a cached-attention block (LN + RoPE + MHA).

Sharding over 8 cores: data-parallel over batch (4) x tensor-parallel over
heads (16 -> 2 groups of 8). Core c handles batch c//2, head-group c%2.
Each core computes a partial output projection (its 8 heads); the two
partials per batch are summed on the host (the all-reduce of the hint).

Per-core dataflow (S=2048, D=1024, 8 local heads, head dim 64):
  Phase 1 (one merged software-pipelined loop over 16 s-chunks):
    LN stats via bn_stats (DVE) + sqrt (ACT) + reciprocal (DVE);
    normalize on ACT (Identity, scale=rstd, bias=-mu*rstd from Pool);
    RoPE on q/k split between DVE (first half) and Pool (second half);
    PE transpose to [d, s]; projections on PE -> qhT/khT [hk, s] fp16 and
    vh [sk, hv] fp16 with a trailing all-ones column per head.  The last
    4-chunk group\'s q/k projections are NOT run here -- they are
    deferred into phase 2.
  Phase 2 (attention, per q-half then head-pair; ACT exp is the
  bottleneck, so spare PE/DVE capacity is used for deferred work
  embedded at skc==1 of each pass, scratching in the ctx PSUM tags
  which are not allocated until skc==2):
    qc==0 passes: the deferred sg3 projections for this head-pair.
    qc==1 passes: the output projections for two of qc0\'s s-chunks.
    scores^T [sk, q] fp32 PSUM = khT.T @ qhT (K=64, two heads row-packed
    via tile_position auto-derive); exp on ACT; ctx^T [65, q] fp32
    accumulated over sk chunks, row 64 = prob sums; reciprocal of sums
    (fast DVE op), partition_broadcast on Pool, normalize during
    evacuation (DVE).
  Phase 3: output projection for the remaining 8 s-chunks.
"""

import numpy as np

S = 2048
D = 1024
H_LOC = 8  # heads per core
HK = H_LOC * 64  # 512
N_CORES = 8
EPS = 1e-6
ROPE_BASE = 10000.0

_cached = {}


def _build_program(dbg=False, loop_k=None, ph=(1, 1, 1, 1)):
    import contextlib

    import concourse.tile as tile
    from concourse import bacc, mybir

    f32 = mybir.dt.float32
    f16 = mybir.dt.float16
    AF = mybir.ActivationFunctionType
    OP = mybir.AluOpType

    nc = bacc.Bacc("TRN2", target_bir_lowering=False, debug=False,
                   num_devices=N_CORES)

    xq_d = nc.dram_tensor("xq", [S, D], f32, kind="ExternalInput").ap()
    xk_d = nc.dram_tensor("xk", [S, D], f32, kind="ExternalInput").ap()
    xv_d = nc.dram_tensor("xv", [S, D], f32, kind="ExternalInput").ap()
    wq_d = nc.dram_tensor("wq", [128, 8, HK], f16, kind="ExternalInput").ap()
    wk_d = nc.dram_tensor("wk", [128, 8, HK], f16, kind="ExternalInput").ap()
    wv_d = nc.dram_tensor("wv", [128, 8, HK], f16, kind="ExternalInput").ap()
    wo_d = nc.dram_tensor("wo", [128, 4, D], f16, kind="ExternalInput").ap()
    cos_d = nc.dram_tensor("cost", [S, D // 2], f16, kind="ExternalInput").ap()
    sin_d = nc.dram_tensor("sint", [S, D // 2], f16, kind="ExternalInput").ap()
    id_d = nc.dram_tensor("ident", [128, 128], f16, kind="ExternalInput").ap()
    out_d = nc.dram_tensor("out", [S, D], f32, kind="ExternalOutput").ap()
    if dbg:
        dbg_qhT = nc.dram_tensor("dbg_qhT", [128, 4, S], f16,
                                 kind="ExternalOutput").ap()
        dbg_khT = nc.dram_tensor("dbg_khT", [128, 4, S], f16,
                                 kind="ExternalOutput").ap()
        dbg_vh = nc.dram_tensor("dbg_vh", [128, 16, H_LOC * 65], f16,
                                kind="ExternalOutput").ap()
        dbg_ctxT = nc.dram_tensor("dbg_ctxT", [128, 4, S], f16,
                                  kind="ExternalOutput").ap()
        dbg_pr = nc.dram_tensor("dbg_pr", [128, 16, 1024], f16,
                                kind="ExternalOutput").ap()
        dbg_qT = nc.dram_tensor("dbg_qT", [128, 8, 512], f16,
                                kind="ExternalOutput").ap()
        dbg_ctxu = nc.dram_tensor("dbg_ctxu", [65, 1024], f32,
                                  kind="ExternalOutput").ap()
        dbg_rs = nc.dram_tensor("dbg_rs", [1, 1024], f32,
                                kind="ExternalOutput").ap()

    with tile.TileContext(nc) as tc:
        with tc.tile_pool(name="persist", bufs=1) as P:
            # --- persistent SBUF ---
            wq_sb = P.tile([128, 8, HK], f16, tag="wq")
            wk_sb = P.tile([128, 8, HK], f16, tag="wk")
            wv_sb = P.tile([128, 8, HK], f16, tag="wv")
            wo_sb = P.tile([128, 4, D], f16, tag="wo")
            nc.sync.dma_start(wq_sb[:], wq_d[:])
            nc.sync.dma_start(wk_sb[:], wk_d[:])
            nc.sync.dma_start(wv_sb[:], wv_d[:])
            nc.sync.dma_start(wo_sb[:], wo_d[:])

            qhT = P.tile([128, 4, S], f16, tag="qhT")   # [hk, s] head h: part 64*(h%2), chunk h//2
            khT = P.tile([128, 4, S], f16, tag="khT")
            vh = P.tile([128, 16, H_LOC * 65], f16, tag="vh")  # [sk, skc, h*65+v]; col h*65+0 = 1
            ctxT = P.tile([128, 4, S], f16, tag="ctxT")
            eps_sb = P.tile([128, 1], f32, tag="eps")
            nc.vector.memset(eps_sb[:], EPS)
            id_sb = P.tile([128, 128], f16, tag="ident")
            nc.sync.dma_start(id_sb[:], id_d[:])
            vh3 = vh[:].rearrange("p c (h u) -> p c h u", u=65)
            for h in range(H_LOC):
                nc.vector.memset(vh3[:, :, h, 64:65], 1.0)

            if not ph[0]:
                # phase-isolated benches: init tensors phase 1 would write
                nc.vector.memset(qhT[:], 0.01)
                nc.vector.memset(khT[:], 0.01)
                nc.vector.memset(ctxT[:], 0.01)

            loop_cm = (tc.For_i(0, loop_k, 1) if loop_k is not None
                       else contextlib.nullcontext())

            with loop_cm, tc.tile_pool(name="slab", bufs=2) as SL:
                slabs = {}
                if ph[0]:
                    # ---------------- Phase 1: merged q/k/v pipeline --------
                    with tc.tile_pool(name="ph1x", bufs=2) as X, \
                         tc.tile_pool(name="ph1", bufs=3) as Q, \
                         tc.tile_pool(name="ps1x", bufs=2, space="PSUM") as PSX, \
                         tc.tile_pool(name="ps1p", bufs=2, space="PSUM") as PSP:

                        def ln_stats(x_t, tag):
                            stats = Q.tile([128, 2, 6], f32, tag=f"st_{tag}",
                                           name="st")
                            mv = Q.tile([128, 2], f32, tag=f"mv_{tag}",
                                        name="mv")
                            sq = Q.tile([128, 1], f32, tag=f"sq_{tag}",
                                        name="sq")
                            rstd = Q.tile([128, 1], f32, tag=f"rs_{tag}",
                                          name="rstd")
                            nmr = Q.tile([128, 1], f32, tag=f"nm_{tag}",
                                         name="nmr")
                            nc.vector.bn_stats(stats[:, 0, :], x_t[:, 0:512])
                            nc.vector.bn_stats(stats[:, 1, :], x_t[:, 512:1024])
                            nc.vector.bn_aggr(mv[:], stats[:])
                            nc.scalar.activation(sq[:], mv[:, 1:2], AF.Sqrt,
                                                 bias=eps_sb[:])
                            nc.vector.reciprocal(rstd[:], sq[:])
                            nc.gpsimd.tensor_scalar(
                                nmr[:], mv[:, 0:1], rstd[:], -1.0,
                                OP.mult, OP.mult)
                            return rstd, nmr

                        def normalize(x_t, out_t, rstd, nmr):
                            nc.scalar.activation(out_t[:], x_t[:], AF.Identity,
                                                 bias=nmr[:], scale=rstd[:])

                        def project(sg):
                            qTg, kTg = slabs.pop(sg)
                            if dbg and sg == 0:
                                nc.sync.dma_start(dbg_qT[:], qTg[:])
                            for hkc in range(4):
                                for w_sb, src, dst in ((wq_sb, qTg, qhT),
                                                       (wk_sb, kTg, khT)):
                                    ps = PSP.tile([128, 512], f32, tag="pj",
                                                  name="pj")
                                    for dc in range(8):
                                        nc.tensor.matmul(
                                            ps[:],
                                            lhsT=w_sb[:, dc, hkc * 128:(hkc + 1) * 128],
                                            rhs=src[:, dc, :],
                                            start=(dc == 0), stop=(dc == 7))
                                    nc.scalar.copy(
                                        dst[:, hkc, sg * 512:(sg + 1) * 512],
                                        ps[:])

                        prev = None
                        for i in range(17):
                            cur = None
                            if i < 16:
                                if i % 4 == 0:
                                    slabs[i // 4] = (
                                        SL.tile([128, 8, 512], f16, tag="qTg",
                                                name="qTg"),
                                        SL.tile([128, 8, 512], f16, tag="kTg",
                                                name="kTg"))
                                cos_t = Q.tile([128, 512], f16, tag="cos")
                                sin_t = Q.tile([128, 512], f16, tag="sin")
                                nc.sync.dma_start(cos_t[:], cos_d[i * 128:(i + 1) * 128, :])
                                nc.sync.dma_start(sin_t[:], sin_d[i * 128:(i + 1) * 128, :])
                                cur = {"cos": cos_t, "sin": sin_t, "i": i}
                                for nm, x_d in (("q", xq_d), ("k", xk_d),
                                                ("v", xv_d)):
                                    x_t = X.tile([128, D], f32, tag=f"x{nm}",
                                                 name="x_t")
                                    nc.sync.dma_start(x_t[:], x_d[i * 128:(i + 1) * 128, :])
                                    rstd, nmr = ln_stats(x_t, nm)
                                    cur[nm] = (x_t, rstd, nmr)
                            if prev is not None:
                                pi = prev["i"]
                                psg, pst = pi // 4, pi % 4
                                # ---- v path ----
                                x_t, rstd, nmr = prev["v"]
                                vn_t = X.tile([128, D], f16, tag="nv",
                                              name="vn_t")
                                normalize(x_t, vn_t, rstd, nmr)
                                xp_ps = PSX.tile([128, 8, 128], f16, tag="xpv",
                                                 name="xp_v")
                                for dc in range(8):
                                    nc.tensor.transpose(
                                        xp_ps[:, dc, :],
                                        vn_t[:, dc * 128:(dc + 1) * 128],
                                        id_sb[:])
                                vT_t = X.tile([128, 8, 128], f16, tag="vT",
                                              name="vT_t")
                                nc.vector.tensor_copy(vT_t[:], xp_ps[:])
                                vp = PSP.tile([128, HK], f32, tag="pj",
                                              name="vp")
                                for dc in range(8):
                                    nc.tensor.matmul(vp[:], lhsT=vT_t[:, dc, :],
                                                     rhs=wv_sb[:, dc, :],
                                                     start=(dc == 0),
                                                     stop=(dc == 7))
                                nc.scalar.copy(
                                    vh3[:, pi, :, 0:64],
                                    vp[:].rearrange("p (h v) -> p h v", v=64))
                                # ---- q/k path ----
                                xps = {}
                                for nm in ("q", "k"):
                                    x_t, rstd, nmr = prev[nm]
                                    n_t = Q.tile([128, D], f16, tag=f"n{nm}",
                                                 name="n_t")
                                    normalize(x_t, n_t, rstd, nmr)
                                    r_t = Q.tile([128, D], f16, tag=f"r{nm}",
                                                 name="r_t")
                                    b = Q.tile([128, 512], f16, tag=f"b{nm}",
                                               name="b")
                                    d = Q.tile([128, 512], f16, tag=f"d{nm}",
                                               name="d")
                                    # first rope half on DVE (in-place sub)
                                    nc.vector.tensor_mul(
                                        r_t[:, 0:512], n_t[:, 0:512],
                                        prev["cos"][:])
                                    nc.vector.tensor_mul(
                                        b[:], n_t[:, 512:1024], prev["sin"][:])
                                    nc.vector.tensor_sub(
                                        r_t[:, 0:512], r_t[:, 0:512], b[:])
                                    # second rope half: muls on Pool,
                                    # final add on DVE
                                    nc.gpsimd.tensor_mul(
                                        r_t[:, 512:1024], n_t[:, 512:1024],
                                        prev["cos"][:])
                                    nc.gpsimd.tensor_mul(
                                        d[:], n_t[:, 0:512], prev["sin"][:])
                                    nc.vector.tensor_add(
                                        r_t[:, 512:1024], r_t[:, 512:1024],
                                        d[:])
                                    xq_ps = PSX.tile([128, 8, 128], f16,
                                                     tag=f"xp{nm}", name="xq_ps")
                                    for dc in range(8):
                                        nc.tensor.transpose(
                                            xq_ps[:, dc, :],
                                            r_t[:, dc * 128:(dc + 1) * 128],
                                            id_sb[:])
                                    xps[nm] = xq_ps
                                for nm, dst_i in (("q", 0), ("k", 1)):
                                    nc.scalar.copy(
                                        slabs[psg][dst_i][:, :, pst * 128:(pst + 1) * 128],
                                        xps[nm][:])
                                if pst == 3 and psg < 3:
                                    # sg3's projections are deferred into
                                    # phase 2 (hidden under ACT-bound passes)
                                    project(psg)
                            prev = cur

                if ph[2]:
                    # ---------------- Phase 2: attention + out-proj ---------
                    import concourse.bass as bass
                    with tc.tile_pool(name="ph2", bufs=10) as PR, \
                         tc.tile_pool(name="ph2b", bufs=2) as RS, \
                         tc.tile_pool(name="ps2", bufs=1, space="PSUM") as PS2:
                        for qc in range(2):
                            q0 = qc * 1024
                            for hp in range(4):
                                h0, h1 = 2 * hp, 2 * hp + 1
                                ctx_ps = None
                                # software pipeline: ctx matmuls for chunk
                                # skc-2 are issued after the scores/exp of
                                # chunk skc, so the PE never sits directly
                                # behind the exp it is waiting for.
                                hist = {}
                                for skc in range(18):
                                    cur = [None, None]
                                    if skc < 16:
                                        st_ps = [PS2.tile([128, 1024], f32, tag=f"sc{e}",
                                                           name=f"sc{e}") for e in range(2)]
                                        for j in range(2):
                                            for e, h in ((0, h0), (1, h1)):
                                                pb = slice(64 * (h % 2), 64 * (h % 2) + 64)
                                                nc.tensor.matmul(
                                                    st_ps[e][:, j * 512:(j + 1) * 512],
                                                    lhsT=khT[pb, hp, skc * 128:(skc + 1) * 128],
                                                    rhs=qhT[pb, hp, q0 + j * 512:q0 + (j + 1) * 512])
                                        for e, h in ((0, h0), (1, h1)):
                                            pr = PR.tile([128, 1024], f16, tag=f"pr{e}")
                                            nc.scalar.activation(pr[:], st_ps[e][:], AF.Exp,
                                                                 scale=0.125)
                                            if dbg and hp == 0 and qc == 0 and e == 0:
                                                nc.sync.dma_start(dbg_pr[:, skc, :], pr[:])
                                            cur[e] = pr
                                        hist[skc] = cur
                                    if skc == 1 and qc == 0 and ph[0]:
                                        # deferred sg3 projection for this
                                        # head-pair (hkc == hp): runs on PE
                                        # under the first exps; scratch in
                                        # the ctx tags (freed by the DVE
                                        # evac before the first ctx matmul
                                        # at skc == 2), evac on DVE
                                        qTg3, kTg3 = slabs[3]
                                        for ee, (w_sb, src3, dst) in enumerate(
                                                ((wq_sb, qTg3, qhT),
                                                 (wk_sb, kTg3, khT))):
                                            pps = PS2.tile([128, 1024], f32,
                                                           tag=f"ctx{ee}",
                                                           name="pj3")
                                            for dc in range(8):
                                                nc.tensor.matmul(
                                                    pps[:, 0:512],
                                                    lhsT=w_sb[:, dc, hp * 128:(hp + 1) * 128],
                                                    rhs=src3[:, dc, :],
                                                    start=(dc == 0),
                                                    stop=(dc == 7))
                                            nc.vector.tensor_copy(
                                                dst[:, hp, 3 * 512:4 * 512],
                                                pps[:, 0:512])
                                    if skc == 1 and qc == 1 and ph[3]:
                                        # output projection for two of
                                        # qc0's s-chunks, hidden under the
                                        # first exps of this pass (ctx-tag
                                        # scratch, freed before skc == 2)
                                        for ee in range(2):
                                            sc_o = hp * 2 + ee
                                            pps = PS2.tile([128, 1024], f32,
                                                           tag=f"ctx{ee}",
                                                           name="oproj")
                                            for hvc in range(4):
                                                for dmc in range(2):
                                                    nc.tensor.matmul(
                                                        pps[:, dmc * 512:(dmc + 1) * 512],
                                                        lhsT=ctxT[:, hvc, sc_o * 128:(sc_o + 1) * 128],
                                                        rhs=wo_sb[:, hvc, dmc * 512:(dmc + 1) * 512],
                                                        start=(hvc == 0),
                                                        stop=(hvc == 3))
                                            o_sb = RS.tile([128, 1024], f32,
                                                           tag="osb2")
                                            nc.vector.tensor_copy(o_sb[:], pps[:])
                                            nc.sync.dma_start(
                                                out_d[sc_o * 128:(sc_o + 1) * 128, :],
                                                o_sb[:])
                                    if skc == 2:
                                        ctx_ps = [PS2.tile([65, 1024], f32,
                                                            tag=f"ctx{e}",
                                                            name=f"ctx{e}")
                                                  for e in range(2)]
                                    pskc = skc - 2
                                    if pskc >= 0:
                                        pcur = hist.pop(pskc)
                                        for e, h in ((0, h0), (1, h1)):
                                            lw = vh3[:, pskc, h, 0:65]
                                            for j in range(2):
                                                nc.tensor.matmul(
                                                    ctx_ps[e][:, j * 512:(j + 1) * 512],
                                                    lhsT=lw,
                                                    rhs=pcur[e][:, j * 512:(j + 1) * 512],
                                                    start=(pskc == 0), stop=(pskc == 15))
                                if dbg and hp == 0 and qc == 0:
                                    ctxu_sb = RS.tile([65, 1024], f32, tag="dbgu")
                                    nc.vector.tensor_copy(ctxu_sb[:], ctx_ps[0][:])
                                    nc.sync.dma_start(dbg_ctxu[:], ctxu_sb[:])
                                for e, h in ((0, h0), (1, h1)):
                                    # sums row (partition 64) staged to
                                    # partition 0 for the custom DVE recip;
                                    # the normalize mul reads ctx PSUM
                                    # directly during evacuation.
                                    s_sb = RS.tile([1, 1024], f32, tag="ssum")
                                    nc.vector.tensor_copy(s_sb[:], ctx_ps[e][64:65, :])
                                    cu = RS.tile([64, 1024], f16, tag="cu")
                                    nc.vector.tensor_copy(cu[:], ctx_ps[e][0:64, :])
                                    rs = RS.tile([1, 1024], f32, tag="rs")
                                    nc.vector.reciprocal_approx_fast(
                                        out=rs[:], in_=s_sb[:])
                                    rs16 = RS.tile([1, 1024], f16, tag="rs16")
                                    nc.vector.tensor_copy(rs16[:], rs[:])
                                    rsb = RS.tile([64, 1024], f16, tag="rsb")
                                    nc.gpsimd.partition_broadcast(
                                        rsb[:], rs16[:], channels=64)
                                    if dbg and hp == 0 and qc == 0 and e == 0:
                                        nc.sync.dma_start(dbg_rs[:], rs[:])
                                    nc.vector.tensor_mul(
                                        ctxT[64 * (h % 2):64 * (h % 2) + 64, h // 2,
                                             q0:q0 + 1024],
                                        cu[:], rsb[:])
                    # ---------------- Phase 3: output projection ----------
                    with tc.tile_pool(name="ph3", bufs=3) as O, \
                         tc.tile_pool(name="ps3", bufs=2, space="PSUM") as PS3:
                        for sc in range(8 if ph[2] else 0, 16):
                            ops = PS3.tile([128, 1024], f32, tag="out",
                                           name="out_ps")
                            for hvc in range(4):
                                for dmc in range(2):
                                    nc.tensor.matmul(
                                        ops[:, dmc * 512:(dmc + 1) * 512],
                                        lhsT=ctxT[:, hvc, sc * 128:(sc + 1) * 128],
                                        rhs=wo_sb[:, hvc, dmc * 512:(dmc + 1) * 512],
                                        start=(hvc == 0), stop=(hvc == 3))
                            o_sb = O.tile([128, 1024], f32, tag="osb")
                            nc.vector.tensor_copy(o_sb[:], ops[:])
                            nc.sync.dma_start(
                                out_d[sc * 128:(sc + 1) * 128, :], o_sb[:])

                    if dbg:
                        nc.sync.dma_start(dbg_qhT[:], qhT[:])
                        nc.sync.dma_start(dbg_khT[:], khT[:])
                        nc.sync.dma_start(dbg_vh[:], vh[:])
                        nc.sync.dma_start(dbg_ctxT[:], ctxT[:])

    nc.compile()
    return nc


def _rope_tables():
    half = D // 2
    inv_freq = (1.0 / (np.float32(ROPE_BASE) **
                       (np.arange(half, dtype=np.float32) / np.float32(half))))
    ang = (np.arange(S, dtype=np.float32)[:, None].astype(np.float32)
           * inv_freq[None, :]).astype(np.float32)
    return (np.cos(ang).astype(np.float16),
            np.sin(ang).astype(np.float16))


def kernel(query, key, value, Wq, bq, Wk, bk, Wv, bv, Wo, bo):
    from concourse.bass_utils import run_bass_kernel_spmd

    if "nc" not in _cached:
        _cached["nc"] = _build_program()
    nc = _cached["nc"]

    cos_t, sin_t = _rope_tables()

    def wlayout(w):  # [1024, 512] -> [128, 8, 512]
        return np.ascontiguousarray(
            w.reshape(8, 128, w.shape[1]).transpose(1, 0, 2)).astype(np.float16)

    in_maps = []
    for c in range(N_CORES):
        b, hg = divmod(c, 2)
        hs = slice(hg * H_LOC, (hg + 1) * H_LOC)
        in_maps.append({
            "xq": np.ascontiguousarray(query[b]).astype(np.float32),
            "xk": np.ascontiguousarray(key[b]).astype(np.float32),
            "xv": np.ascontiguousarray(value[b]).astype(np.float32),
            "wq": wlayout(np.asarray(Wq)[:, hs, :].reshape(D, HK)),
            "wk": wlayout(np.asarray(Wk)[:, hs, :].reshape(D, HK)),
            "wv": wlayout(np.asarray(Wv)[:, hs, :].reshape(D, HK)),
            "wo": np.ascontiguousarray(
                np.asarray(Wo)[hs].reshape(HK, D).reshape(4, 128, D)
                .transpose(1, 0, 2)).astype(np.float16),
            "cost": cos_t,
            "sint": sin_t,
            "ident": np.eye(128, dtype=np.float16),
        })

    _cached["in_maps"] = in_maps
    res = run_bass_kernel_spmd(nc, in_maps, core_ids=list(range(N_CORES)))
    outs = [r["out"] for r in res.results]
    full = np.stack([outs[2 * b] + outs[2 * b + 1] for b in range(4)])
    full = full + np.asarray(bo, dtype=np.float32)[None, None, :]
    return full.astype(np.float32)
